# revision 47
# baseline (speedup 1.0000x reference)
"""Trainium2 Bass kernel for nn_C_Aggregation_24807731101830.

Patch-embed conv (stride 16 = kernel 16) + sequential Gauss-Seidel-like
index-update scan over a flattened 34x34 grid, batch-sharded over 8 cores.

v11 (lane split DVE/PE): the scan is a fixed linear operator L on the
conv output (computed exactly on the host by pushing basis vectors
through the reference scan). Channels 0-511 (8 of 12 lane-groups) run
the fp16 DVE multiply-form scan; channels 512-767 skip the scan: a
transposed conv (xt_T[q, c], q on partitions) feeds a block-banded PE
matmul out_T = L_io @ conv_T (24 nonzero 128x128 blocks; L_io maps the
1024 interior conv positions to ALL 1156 output positions, so the
scan-updated grid-border positions are handled too — identity rows cover
untouched spots). The rank-1 beta*bias term and the 4-position border
tail are added on the host.

DVE path (8 segments): multiply-form scan state=(d0+state)*M with
M=[1,.125*31,0], borders on DVE during the fill, writeback on ACT,
octet-aligned band pieces (4-tap of orig, first pass on GpSimd).
"""
import sys
import types
import numpy as np

import concourse.mybir as mybir
from concourse import bass, tile
from concourse.bass_utils import run_bass_kernel_spmd
from contextlib import ExitStack

F32 = mybir.dt.float32
F16 = mybir.dt.float16
AOP = mybir.AluOpType
IDENT = mybir.ActivationFunctionType.Identity

N_CORES = 8
B_LOC = 2            # batches per core
CG_D = 2             # channel groups on the DVE scan path
NBG = B_LOC * CG_D   # 8 scan lane-groups
Q34 = 1156           # 34*34
QF = NBG * Q34

LAST_EXEC_NS = None

# interior grid (gi,gj) in 1..32 -> flat 34*gi+gj, row-major q index
INT_FLAT = np.array([34 * gi + gj for gi in range(1, 33)
                     for gj in range(1, 33)])


def _compute_L():
    """Exact linear operator of the reference scan (f64 basis propagation):
    L_io maps interior conv values to all 1156 outputs (padded to 9x128 =
    1152 output rows; flat 1152..1155 is border-row tail, bias-only)."""
    N, pn = Q34, 32
    buf = np.eye(N, dtype=np.float64)
    for i in range(1, pn - 1):
        for j in range(1, pn):
            idx = i * pn + j
            s = (buf[:, idx - pn] + buf[:, idx + pn] + buf[:, idx - 1]
                 + buf[:, idx + 1] + buf[:, idx - pn - 1]
                 + buf[:, idx - pn + 1] + buf[:, idx + pn - 1]
                 + buf[:, idx + pn + 1]) / 8.0
            buf[:, idx] = s
    Lmat = buf.T
    L_io = np.zeros((1152, 1024))
    L_io[:] = Lmat[:1152, INT_FLAT]
    beta = Lmat[:1152, :].sum(axis=1)
    T = L_io.reshape(9, 128, 8, 128)
    blocks = [(tp, t) for tp in range(9) for t in range(8)
              if np.abs(T[tp, :, t, :]).max() > 2e-4]
    Lb = np.stack([np.ascontiguousarray(T[tp, :, t, :].T)
                   for tp, t in blocks]).astype(np.float16)
    return Lb, blocks, beta.astype(np.float32)


L_BLOCKS, BLOCK_LIST, BETA = None, None, None


def _install_ntff_hook():
    try:
        import trn_agent_boot.trn_boot as tb
        mod = types.ModuleType("antenv.axon_hooks")
        holder = [None]
        mod.set_axon_ntff_profile_hook = lambda h: holder.__setitem__(0, h)
        mod.get_axon_ntff_profile_hook = lambda: holder[0]
        sys.modules["antenv.axon_hooks"] = mod
        import antenv
        antenv.axon_hooks = mod
        mod.set_axon_ntff_profile_hook(
            tb._ntff_profile_via_ctypes('/opt/axon/libaxon_pjrt.so'))
        return True
    except Exception:
        return False


def _split_sp_multiwaits(nc):
    cnt = 0
    for f in nc.m.functions:
        for blk in f.blocks:
            insts = blk.instructions
            i = 0
            while i < len(insts):
                inst = insts[i]
                si = getattr(inst, 'sync_info', None)
                if (getattr(inst, 'engine', None) is not None
                        and si is not None and si.on_wait and len(si.on_wait) > 1):
                    waits = list(si.on_wait)
                    new = []
                    for w in waits[:-1]:
                        nop = mybir.InstNoOp(name=f"mwfix-{inst.name}-{cnt}",
                                             ins=[], outs=[])
                        cnt += 1
                        nop.engine = inst.engine
                        nop.sync_info = mybir.SyncInfo(on_wait=[w], on_update=[])
                        new.append(nop)
                    inst.sync_info = mybir.SyncInfo(
                        on_wait=[waits[-1]], on_update=list(si.on_update or []))
                    insts[i:i] = new
                    i += len(new)
                i += 1
    return cnt


PIECES = [(1, 7), (8, 16), (17, 24), (25, 30)]
CHUNKS = [(0, 512, 15), (512, 737, 22), (993, 1156, -3),
          (737, 897, 27), (897, 961, 29), (961, 993, 30)]
# stage-2 output tiles emitted after each octet (their L-band inputs ready)
ST2_SCHED = {0: [0, 1], 1: [2, 3], 2: [4, 5], 3: [6, 7, 8]}


def _build(block_list):
    nblk = len(block_list)
    nc = bass.Bass("TRN2", target_bir_lowering=False)
    xP_d = nc.declare_dram_parameter("xP", [768, B_LOC, 1024], F16, isOutput=False)
    wT_d = nc.declare_dram_parameter("wT", [768, 768], F16, isOutput=False)
    bias_d = nc.declare_dram_parameter("bias", [768], F32, isOutput=False)
    Lb_d = nc.declare_dram_parameter("Lb", [nblk, 128, 128], F16, isOutput=False)
    xf_d = nc.declare_dram_parameter("xf", [B_LOC, 256, Q34], F16, isOutput=True)
    xt2_d = nc.declare_dram_parameter("xt2", [B_LOC, 1152, 512], F16,
                                      isOutput=True)
    xfr = xf_d.rearrange("b (g p) q -> p b g q", p=128)

    with tile.TileContext(nc) as tc, ExitStack() as ctx:
        sb = ctx.enter_context(tc.tile_pool(name="sb", bufs=1))
        ps = ctx.enter_context(tc.tile_pool(name="ps", bufs=4, space="PSUM"))
        pbpool = ctx.enter_context(tc.tile_pool(name="pb", bufs=2))
        o2pool = ctx.enter_context(tc.tile_pool(name="o2", bufs=3))

        # preload ACT function table
        scr = sb.tile([128, 8], F16, tag="scr")
        nc.scalar.activation(scr[:], scr[:], IDENT)

        # ---- input loads: bias/weights/L + batch-1 x on ACT queue,
        #      batch-0 x on SP ----
        biast = sb.tile([128, 6], F32, tag="bias")
        nc.scalar.dma_start(biast[:], bias_d.rearrange("(a p) -> p a", p=128))
        wt = sb.tile([128, 6, 768], F16, tag="wt")
        wTr = wT_d.rearrange("(a p) c -> p a c", p=128)
        for h in range(2):
            nc.scalar.dma_start(wt[:, 3 * h:3 * h + 3, :],
                                wTr[:, 3 * h:3 * h + 3, :])
        xpt = sb.tile([128, 6, B_LOC * 1024], F16, tag="xpt")
        xPr = xP_d.rearrange("(a p) b q -> p a b q", p=128)
        xpt4 = xpt[:].rearrange("p a (b q) -> p a b q", b=B_LOC)
        for q0, q1 in [(0, 128), (128, 256), (256, 512), (512, 768),
                       (768, 1024)]:
            for b in range(B_LOC):
                eng = nc.sync if b == 0 else nc.scalar
                eng.dma_start(xpt4[:, :, b:b + 1, q0:q1],
                              xPr[:, :, b:b + 1, q0:q1])
        Lt = sb.tile([128, nblk, 128], F16, tag="Lt")
        nc.scalar.dma_start(Lt[:], Lb_d.rearrange("k p c -> p k c"))

        # ---- constants ----
        mmask = sb.tile([128, NBG * 33], F16, tag="mmask")
        nc.vector.memset(mmask[:], 0.125)
        mm3 = mmask[:].rearrange("p (g c) -> p g c", g=NBG)
        nc.vector.memset(mm3[:, :, 0:1], 1.0)
        nc.vector.memset(mm3[:, :, 32:33], 0.0)
        zt = sb.tile([128, 64], F16, tag="zt")
        nc.vector.memset(zt[:], 0.0)

        # ---- DVE-path buf (channels 0..511) ----
        buf = sb.tile([128, QF], F16, tag="buf")
        buf3 = buf[:].rearrange("p (bg q) -> p bg q", bg=NBG)
        buf4 = buf[:].rearrange("p (bg gi gj) -> p bg gi gj", bg=NBG, gi=34)

        ua = sb.tile([128, NBG * 31], F16, tag="ua")
        uav = ua[:].rearrange("p (g c) -> p g c", g=NBG)
        ub = sb.tile([128, NBG * 31], F16, tag="ub")
        ubv = ub[:].rearrange("p (g c) -> p g c", g=NBG)
        d0t = [sb.tile([128, NBG * 33], F16, tag=f"d0_{k}", name=f"d0_{k}")
               for k in range(2)]
        st = [sb.tile([128, NBG * 33], F16, tag=f"s_{k}", name=f"s_{k}")
              for k in range(8)]
        for k in range(2):
            nc.vector.memset(
                d0t[k][:].rearrange("p (g c) -> p g c", g=NBG)[:, :, 32:33], 0.0)
        # transposed conv results for the PE path: [q 128, (b, qtile), c 256]
        xtT = sb.tile([128, B_LOC, 8, 512], F16, tag="xtT")

        # ---- borders = bias on DVE (idle during fill) ----
        for b in range(B_LOC):
            for m in range(CG_D):
                bg = b * CG_D + m
                bcol = biast[:, m:m + 1]
                views = [buf3[:, bg, 0:35],
                         buf3[:, bg:bg + 1, 67:67 + 34 * 31].rearrange(
                             "p o (r t) -> p (o r) t", t=34)[:, :, 0:2],
                         buf3[:, bg, 1121:1156]]
                zins = [zt[:, 0:35],
                        zt[:, 0:62].rearrange("p (r t) -> p r t", t=2),
                        zt[:, 0:35]]
                for v, z in zip(views, zins):
                    nc.vector.tensor_scalar(v, z, bcol, None, op0=AOP.add)

        def band_view(base, nrows):
            return buf3[:, :, base:base + 32 * nrows].rearrange(
                "p g (r t) -> p r g t", t=32)[:, :, :, 0:31]

        pb_piece = {}

        def emit_band(piece):
            i0, i1 = PIECES[piece]
            nr = i1 - i0 + 1
            base = 32 * i0 + 2
            pb = pbpool.tile([128, nr * NBG * 31], F16, tag="PB",
                             name=f"pb_{piece}")
            pb4 = pb[:].rearrange("p (r g j) -> p r g j", r=nr, g=NBG)
            eng0 = nc.vector if piece == 0 else nc.gpsimd
            eng0.tensor_tensor(pb4, band_view(base, nr),
                               band_view(base + 30, nr), AOP.add)
            tmp = pbpool.tile([128, nr * NBG * 31], F16, tag="PTMP",
                              name=f"ptmp_{piece}")
            tmp4 = tmp[:].rearrange("p (r g j) -> p r g j", r=nr, g=NBG)
            nc.vector.tensor_tensor(tmp4, band_view(base + 31, nr),
                                    band_view(base + 32, nr), AOP.add)
            nc.vector.tensor_tensor(pb[:], pb[:], tmp[:], AOP.add)
            f0 = i0 if i0 > 1 else 2
            if f0 <= i1:
                nfix = i1 - f0 + 1
                fix_dst = pb4[:, f0 - i0:, :, 30:31]
                fix_src = buf3[:, :, 32 * f0:32 * f0 + 32 * nfix].rearrange(
                    "p g (r t) -> p r g t", t=32)[:, :, :, 0:1]
                nc.vector.scalar_tensor_tensor(
                    fix_dst, fix_src, 1.0, fix_dst, AOP.mult, AOP.add)
            pb_piece[piece] = (pb, i0)

        s_prev_box = [None]

        def emit_row(i):
            qi = 32 * i
            piece = next(p for p, (a, b) in enumerate(PIECES) if a <= i <= b)
            pb, i0 = pb_piece[piece]
            pbr = pb[:].rearrange("p (r g j) -> p r g j",
                                  r=PIECES[piece][1] - i0 + 1,
                                  g=NBG)[:, i - i0, :, :]
            if s_prev_box[0] is None:
                sp = buf3[:, :, 0:33]
            else:
                sp = s_prev_box[0][:].rearrange("p (g c) -> p g c", g=NBG)
            nc.vector.tensor_tensor(uav, sp[:, :, 0:31], sp[:, :, 2:33],
                                    AOP.add)
            nc.vector.tensor_tensor(ubv, sp[:, :, 1:32], pbr, AOP.add)
            d0 = d0t[i % 2]
            d3 = d0[:].rearrange("p (g c) -> p g c", g=NBG)
            nc.vector.tensor_tensor(d3[:, :, 1:32], uav, ubv, AOP.add)
            nc.vector.tensor_scalar(d3[:, :, 0:1], buf3[:, :, qi:qi + 1],
                                    1.0, None, op0=AOP.mult)
            s_cur = st[i % 8]
            nc.vector.tensor_tensor_scan(s_cur[:], d0[:], mmask[:], 0.0,
                                         AOP.add, AOP.mult)
            nc.scalar.mul(
                buf3[:, :, qi + 1:qi + 32],
                s_cur[:].rearrange("p (g c) -> p g c", g=NBG)[:, :, 1:32],
                1.0)
            s_prev_box[0] = s_cur

        def emit_conv(b, gi0, ngi):
            off = b * 1024 + gi0 * 32
            n = ngi * 32
            for m in range(CG_D):
                pt = ps.tile([128, n], F32, tag="ps8",
                             name=f"pt_{b}_{gi0}_{m}", bufs=4)
                for a in range(6):
                    nc.tensor.matmul(
                        pt[:],
                        lhsT=wt[:, a, 128 * m:128 * (m + 1)],
                        rhs=xpt[:, a, off:off + n],
                        start=(a == 0), stop=(a == 5))
                dst = buf4[:, b * CG_D + m, 1 + gi0:1 + gi0 + ngi, 1:33]
                nc.scalar.activation(
                    dst, pt[:].rearrange("p (gi gj) -> p gi gj", gi=ngi),
                    IDENT, bias=biast[:, m:m + 1])

        def emit_stage1T(b, t):
            # xt_T[q, c] for qtile t, channels 512..767 (conv only, no bias)
            pt1 = ps.tile([128, 512], F32, tag="pt1",
                          name=f"p1_{b}_{t}", bufs=2)
            for a in range(6):
                nc.tensor.matmul(
                    pt1[:],
                    lhsT=xpt[:, a, b * 1024 + 128 * t:b * 1024 + 128 * t + 128],
                    rhs=wt[:, a, 256:768],
                    start=(a == 0), stop=(a == 5))
            nc.scalar.mul(xtT[:, b, t, :], pt1[:], 1.0)

        def emit_stage2(b, tp):
            pt2 = ps.tile([128, 512], F32, tag="pt2",
                          name=f"p2_{b}_{tp}", bufs=2)
            blks = [k for k, (tpp, _) in enumerate(BLOCK_LIST) if tpp == tp]
            for n, k in enumerate(blks):
                nc.tensor.matmul(pt2[:], lhsT=Lt[:, k, :],
                                 rhs=xtT[:, b, BLOCK_LIST[k][1], :],
                                 start=(n == 0), stop=(n == len(blks) - 1))
            o2 = o2pool.tile([128, 512], F16, tag="o2", name=f"o2_{b}_{tp}")
            nc.scalar.mul(o2[:], pt2[:], 1.0)
            dst = xt2_d[b:b + 1, 128 * tp:128 * (tp + 1), :].rearrange(
                "b q c -> q (b c)")
            nc.sync.dma_start(dst, o2[:])

        def emit_out_chunk(lo, hi):
            nc.sync.dma_start(
                xfr[:, :, :, lo:hi],
                buf3[:, :, lo:hi].rearrange("p (b g) q -> p b g q", b=B_LOC))

        chunk_after = {r: (lo, hi) for lo, hi, r in CHUNKS}
        CONV_PARTS = [(0, 8), (8, 8), (16, 8), (24, 8)]
        for piece, (gi0, ngi) in enumerate(CONV_PARTS):
            emit_conv(0, gi0, ngi)
            emit_conv(1, gi0, ngi)
            for b in range(B_LOC):
                emit_stage1T(b, 2 * piece)
                emit_stage1T(b, 2 * piece + 1)
            emit_band(piece)
            for tp in ST2_SCHED[piece]:
                for b in range(B_LOC):
                    emit_stage2(b, tp)
            if piece == 3:
                emit_out_chunk(*chunk_after[-3])
            i0, i1 = PIECES[piece]
            for i in range(i0, i1 + 1):
                emit_row(i)
                if i in chunk_after:
                    emit_out_chunk(*chunk_after[i])

    _split_sp_multiwaits(nc)
    return nc


_NC = None


def kernel(x: np.ndarray, w: np.ndarray, b: np.ndarray) -> np.ndarray:
    global _NC, LAST_EXEC_NS, L_BLOCKS, BLOCK_LIST, BETA
    B, C, H, _ = x.shape          # 16, 3, 512, 512
    assert (B, C, H) == (16, 3, 512)

    if L_BLOCKS is None:
        L_BLOCKS, BLOCK_LIST, BETA = _compute_L()

    xp = x.reshape(B, 3, 32, 16, 32, 16)               # b c gi py gj px
    xp = np.ascontiguousarray(
        xp.transpose(1, 3, 5, 0, 2, 4)).reshape(768, B, 1024)
    xp = xp.astype(np.float16)
    wT = np.ascontiguousarray(w.reshape(768, 768).T).astype(np.float16)
    bias32 = np.ascontiguousarray(b, dtype=np.float32)

    if _NC is None:
        _NC = _build(BLOCK_LIST)

    trace = _install_ntff_hook()
    in_maps = [{"xP": np.ascontiguousarray(xp[:, 2 * r:2 * r + 2, :]),
                "wT": wT, "bias": bias32, "Lb": L_BLOCKS}
               for r in range(N_CORES)]
    try:
        res = run_bass_kernel_spmd(_NC, in_maps, core_ids=list(range(N_CORES)),
                                   trace=trace)
    except Exception:
        if not trace:
            raise
        res = run_bass_kernel_spmd(_NC, in_maps, core_ids=list(range(N_CORES)),
                                   trace=False)
    LAST_EXEC_NS = res.exec_time_ns
    globals()['LAST_RESULT'] = res

    xf = np.concatenate([res.results[r]["xf"] for r in range(N_CORES)], axis=0)
    xt2 = np.concatenate([res.results[r]["xt2"] for r in range(N_CORES)],
                         axis=0)
    # PE-path channels: device L@conv (flat 0..1151), plus the rank-1
    # beta*bias term on the host; flat 1152..1155 is border tail = bias
    pe = np.swapaxes(xt2.astype(np.float32), 1, 2)      # [B, 256, 1152]
    pe += BETA[None, None, :] * bias32[256:, None]
    full = np.empty((B, 768, Q34), np.float32)
    full[:, :256] = xf
    full[:, 256:, :1152] = pe
    full[:, 256:, 1152:] = bias32[256:, None]
    out = full.reshape(B, 3, 544, 544)[:, :, 16:528, 16:528]
    return np.ascontiguousarray(out)
